# revision 16
# baseline (speedup 1.0000x reference)
"""Causal self-attention Trainium2 kernel (B=4, T=2048, C=1024, H=16, D=64).

Sharding: 8 cores = 4 batches x 2 causally-balanced query shards.
Core c handles batch b=c//2 and the 8 interleaved query blocks
g = 2*i + (c%2), i in 0..7 (block = 128 rows).  Every core computes full-
sequence K/V for its batch plus Q for its own query rows, runs all 16 heads
of attention for those rows, and the full output projection for them, so
per-core outputs are disjoint row-slices of y: no collectives, no host math.
One program serves both query parities: the per-kb first-block mask input
(tri / zeros / ones) encodes the parity-dependent causal structure.

Device-side dataflow (matmuls fp16 in / fp32 PSUM accumulate).  All matmul
streams are >=~512 columns so LDWEIGHTS hides behind the moving-tensor
stream (a previous 65/128-col structure was weight-load bound):
  Q^T,K^T = w^T @ x^T      (w stationary; [d, t] layout, head pairs stacked)
  V       = x^T.T @ w_v    (x^T chunks stationary, w_v moving -> V in
                            natural [t, c] layout, no PE transpose)
  S^T[kb] = K[kb] @ Q^T    (K block stationary, streams the whole causally
                            valid q-tail; two heads on partition halves)
  P^T     = exp(0.125*S^T) (ACT, psum->sbuf fp16), data mask on the first
                            valid q-block per kb
  Y^T     = [V|1].T @ P^T  (V natural stationary; accumulates [65, 1024]
                            over kb; row 64 = softmax denominators)
  YT      = Y^T * bcast(1/row64)  (PE ones-matmul broadcast + DVE mul)
  Z       = YT.T @ w_out + ones x b_out  (bias as a K=1 matmul)
"""

import os

import numpy as np

B, T, C = 4, 2048, 1024
H, D = 16, 64
N_CORES = 8
P = 128
QB = 8  # local query blocks per core (of 128 rows)
KB = 16  # key blocks per sequence
PAIRS = 8  # head pairs

_COMPILED = None
LAST_EXEC_NS = None
LAST_TRACE_PATH = None

dbg_stage = os.environ.get("KERNEL_DEBUG_STAGE", "")


def _get_mybir():
    import concourse.mybir as mybir
    return mybir


def split_sync_waits(nc):
    mybir = _get_mybir()
    # This walrus build rejects instructions carrying more than one sync
    # wait (or update).  Split the extras onto NOP carriers: waits go on
    # NOPs inserted before the instruction (same engine, so they gate it),
    # updates onto NOPs after it (fire once it has completed).
    uid = [0]

    def carrier(engine, wait=None, update=None):
        uid[0] += 1
        n = mybir.InstNoOp(
            name=f"I-syncsplit-{uid[0]}",
            opcode="NoOp",
            ins=[],
            outs=[],
            sync_info=mybir.SyncInfo(
                on_wait=[wait] if wait else [],
                on_update=[update] if update else [],
            ),
        )
        n.engine = engine
        return n

    for f in nc.m.functions:
        for blk in f.blocks:
            out = []
            changed = False
            for inst in blk.instructions:
                si = inst.sync_info
                if si is None or (
                    len(si.on_wait) <= 1 and len(si.on_update) <= 1
                ):
                    out.append(inst)
                    continue
                changed = True
                waits = list(si.on_wait)
                updates = list(si.on_update)
                for w in waits[1:]:
                    out.append(carrier(inst.engine, wait=w))
                inst.sync_info = mybir.SyncInfo(
                    on_wait=waits[:1], on_update=updates[:1]
                )
                out.append(inst)
                for u in updates[1:]:
                    out.append(carrier(inst.engine, update=u))
            if changed:
                blk.instructions = out


def _build():
    import concourse.bass as bass
    import concourse.tile as tile
    import concourse.mybir as mybir
    from contextlib import ExitStack

    f32 = mybir.dt.float32
    f16 = mybir.dt.float16
    AF = mybir.ActivationFunctionType

    nc = bass.Bass(
        "TRN2", target_bir_lowering=False, debug=False, num_devices=N_CORES
    )

    xT_d = nc.dram_tensor("xt", [C, T], f16, kind="ExternalInput").ap()
    xqT_d = nc.dram_tensor("xqt", [C, QB * P], f16, kind="ExternalInput").ap()
    wqkv_d = nc.dram_tensor("wqkv", [16, P, 8, P], f16, kind="ExternalInput").ap()
    wv_d = nc.dram_tensor("wv", [P, 8, 1024], f16, kind="ExternalInput").ap()
    wout_d = nc.dram_tensor("wout", [P, 8, C], f16, kind="ExternalInput").ap()
    bqkv_d = nc.dram_tensor("bqkv", [P, 16], f32, kind="ExternalInput").ap()
    bv_d = nc.dram_tensor("bv128", [P, 1024], f16, kind="ExternalInput").ap()
    bout_d = nc.dram_tensor("bout", [1, C], f16, kind="ExternalInput").ap()
    msk_d = nc.dram_tensor("msk", [P, KB, P], f16, kind="ExternalInput").ap()
    ones_d = nc.dram_tensor("ones1", [1, P], f16, kind="ExternalInput").ap()
    yr_d = nc.dram_tensor("yr", [P, QB, C], f32, kind="ExternalOutput").ap()
    dbg_d = {}
    if dbg_stage in ("proj", "vn", "attn"):
        dbg_d["kt"] = nc.dram_tensor(
            "kt", [P, PAIRS, T], f16, kind="ExternalOutput"
        ).ap()
        dbg_d["qt"] = nc.dram_tensor(
            "qt", [P, PAIRS, QB * P], f16, kind="ExternalOutput"
        ).ap()
    if dbg_stage in ("vn", "attn"):
        dbg_d["vn"] = nc.dram_tensor(
            "vn", [P, PAIRS, KB, 2, 65], f16, kind="ExternalOutput"
        ).ap()
    if dbg_stage == "attn":
        dbg_d["yt"] = nc.dram_tensor(
            "yt", [P, 8, QB * P], f16, kind="ExternalOutput"
        ).ap()

    with tile.TileContext(nc) as tc, ExitStack() as ctx:
        persist = ctx.enter_context(tc.tile_pool(name="persist", bufs=1))
        KT = persist.tile([P, PAIRS, T], f16)
        QT = persist.tile([P, PAIRS, QB * P], f16)
        YT = persist.tile([P, 8, QB * P], f16)
        VN = persist.tile([P, PAIRS, KB, 2, 65], f16)
        WO = persist.tile([P, 8, C], f16)
        msk = persist.tile([P, KB, P], f16)
        bqs = persist.tile([P, 16], f32)
        bv = persist.tile([P, 1024], f16)
        on1 = persist.tile([1, P], f16)
        bo = persist.tile([1, C], f16)

        # small persistent inputs first (cheap), then the first Q-proj
        # weight block + XQT so the first matmul can start early.
        nc.sync.dma_start(out=bqs, in_=bqkv_d)
        nc.sync.dma_start(out=msk, in_=msk_d)
        nc.sync.dma_start(out=on1, in_=ones_d)
        nc.sync.dma_start(out=bo, in_=bout_d)
        nc.sync.dma_start(out=bv, in_=bv_d)
        nc.vector.memset(VN[:, :, :, :, 64:65], 1.0)

        xT_v = xT_d.rearrange("(cb p) t -> p cb t", p=P)
        xqT_v = xqT_d.rearrange("(cb p) t -> p cb t", p=P)

        wpool = ctx.enter_context(tc.tile_pool(name="w", bufs=3))
        wt0 = wpool.tile([P, 8, P], f16, tag="w")
        nc.sync.dma_start(out=wt0, in_=wqkv_d[0])

        with (
            tc.tile_pool(name="xt", bufs=1) as xtpool,
            tc.tile_pool(name="xqt", bufs=1) as xqtpool,
            tc.tile_pool(name="wv", bufs=1) as wvpool,
            tc.tile_pool(name="psproj", bufs=3, space="PSUM") as pspool,
        ):
            XQT = xqtpool.tile([P, 8, QB * P], f16)
            for cb in range(8):
                nc.sync.dma_start(out=XQT[:, cb], in_=xqT_v[:, cb])
            XT = xtpool.tile([P, 8, T], f16)
            for cb in range(8):
                nc.sync.dma_start(out=XT[:, cb], in_=xT_v[:, cb])
            WV = wvpool.tile([P, 8, 1024], f16)
            nc.sync.dma_start(out=WV, in_=wv_d)
            nc.sync.dma_start(out=WO, in_=wout_d)

            # Q^T projection (j-blocks 0..7): w stationary, XQT moving
            for pb in range(PAIRS):
                if pb == 0:
                    wt = wt0
                else:
                    wt = wpool.tile([P, 8, P], f16, tag="w")
                    nc.sync.dma_start(out=wt, in_=wqkv_d[pb])
                for t4 in range(2):
                    ps = pspool.tile([P, 512], f32, tag="proj")
                    for cb in range(8):
                        nc.tensor.matmul(
                            ps,
                            lhsT=wt[:, cb],
                            rhs=XQT[:, cb, 512 * t4 : 512 * (t4 + 1)],
                            start=(cb == 0),
                            stop=(cb == 7),
                        )
                    nc.vector.tensor_scalar_add(
                        QT[:, pb, 512 * t4 : 512 * (t4 + 1)],
                        ps,
                        bqs[:, pb : pb + 1],
                    )

            # K^T projection (j-blocks 8..15) over full T
            for pb in range(PAIRS):
                wt = wpool.tile([P, 8, P], f16, tag="w")
                nc.sync.dma_start(out=wt, in_=wqkv_d[8 + pb])
                for t4 in range(4):
                    ps = pspool.tile([P, 512], f32, tag="proj")
                    for cb in range(8):
                        nc.tensor.matmul(
                            ps,
                            lhsT=wt[:, cb],
                            rhs=XT[:, cb, 512 * t4 : 512 * (t4 + 1)],
                            start=(cb == 0),
                            stop=(cb == 7),
                        )
                    nc.vector.tensor_scalar_add(
                        KT[:, pb, 512 * t4 : 512 * (t4 + 1)],
                        ps,
                        bqs[:, 8 + pb : 9 + pb],
                    )

            # V in natural [t, c] layout: x^T chunk stationary, w_v moving
            if dbg_stage != "proj":
                for kb in range(KB):
                    for half in range(2):
                        ps = pspool.tile([P, 512], f32, tag="proj")
                        for cb in range(8):
                            nc.tensor.matmul(
                                ps,
                                lhsT=XT[:, cb, kb * P : (kb + 1) * P],
                                rhs=WV[:, cb, 512 * half : 512 * (half + 1)],
                                start=(cb == 0),
                                stop=(cb == 7),
                            )
                        nc.vector.tensor_add(
                            out=VN[:, 4 * half : 4 * half + 4, kb, :, 0:64],
                            in0=ps.rearrange("p (a b c) -> p a b c", b=2, c=64),
                            in1=bv[:, 512 * half : 512 * (half + 1)].rearrange(
                                "p (a b c) -> p a b c", b=2, c=64
                            ),
                        )

        if "kt" in dbg_d:
            nc.sync.dma_start(out=dbg_d["kt"], in_=KT)
            nc.sync.dma_start(out=dbg_d["qt"], in_=QT)
        if "vn" in dbg_d:
            nc.sync.dma_start(out=dbg_d["vn"], in_=VN)

        # attention: per (pb): S^T per key block over the causally valid
        # q-tail, exp, first-block data mask, then V-stationary accumulation
        # of Y^T[65, 1024] (row 64 = softmax sums), normalized into YT.
        attn_pairs = range(PAIRS) if dbg_stage in ("", "attn", "full") else []
        with (
            tc.tile_pool(name="pts", bufs=2) as ptpool,
            tc.tile_pool(name="sm", bufs=2) as smpool,
            tc.tile_pool(name="psS", bufs=2, space="PSUM") as psSpool,
            tc.tile_pool(name="psY", bufs=1, space="PSUM") as psYpool,
        ):
            for pb in attn_pairs:
                psYT = [
                    psYpool.tile([65, QB * P], f32, tag=f"y{h}", name=f"psYT{h}")
                    for h in range(2)
                ]
                for kb in range(KB):
                    i_min = kb // 2
                    q0 = i_min * P
                    chunks = (
                        [(q0, 512), (512, 1024)] if q0 < 512 else [(q0, 1024)]
                    )
                    pts = {}
                    for (c0, c1) in chunks:
                        cw = c1 - c0
                        for h in range(2):
                            psS = psSpool.tile(
                                [P, 512], f32, tag=f"s{h}", name=f"psS{h}"
                            )[:, :cw]
                            nc.tensor.matmul(
                                psS,
                                lhsT=KT[
                                    64 * h : 64 * h + 64,
                                    pb,
                                    kb * P : (kb + 1) * P,
                                ],
                                rhs=QT[64 * h : 64 * h + 64, pb, c0:c1],
                                start=True,
                                stop=True,
                            )
                            pt = ptpool.tile(
                                [P, 512], f16, tag=f"pt{h}", name=f"pt{h}"
                            )[:, :cw]
                            nc.scalar.activation(pt, psS, AF.Exp, scale=0.125)
                            if c0 <= q0 < c1:
                                nc.vector.tensor_mul(
                                    out=pt[:, q0 - c0 : q0 - c0 + P],
                                    in0=pt[:, q0 - c0 : q0 - c0 + P],
                                    in1=msk[:, kb, :],
                                )
                            pts[(c0, h)] = pt
                    for (c0, c1) in chunks:
                        last = (kb == 7) if c1 <= 512 else (kb == KB - 1)
                        for h in range(2):
                            nc.tensor.matmul(
                                psYT[h][:, c0:c1],
                                lhsT=VN[:, pb, kb, h],
                                rhs=pts[(c0, h)],
                                start=(kb == 0),
                                stop=last,
                                skip_group_check=True,
                            )
                for h in range(2):
                    r = smpool.tile([1, QB * P], f16, tag=f"r{h}")
                    with nc.allow_low_precision(reason="1/softmax-sum in fp16"):
                        nc.vector.reciprocal(r, psYT[h][64:65, :])
                    for half in range(2):
                        rb = psSpool.tile(
                            [P, 512], f32, tag=f"s{h}", name=f"rb{h}"
                        )
                        nc.tensor.matmul(
                            rb[0:64, :],
                            lhsT=on1[:, 0:64],
                            rhs=r[:, 512 * half : 512 * (half + 1)],
                            start=True,
                            stop=True,
                        )
                        rs = smpool.tile([64, 512], f16, tag=f"rs{h}")
                        nc.scalar.copy(rs, rb[0:64, :])
                        nc.vector.tensor_mul(
                            out=YT[
                                64 * h : 64 * h + 64,
                                pb,
                                512 * half : 512 * (half + 1),
                            ],
                            in0=psYT[h][0:64, 512 * half : 512 * (half + 1)],
                            in1=rs,
                        )

        if "yt" in dbg_d:
            nc.sync.dma_start(out=dbg_d["yt"], in_=YT)
        if dbg_stage:
            nc.vector.memset(YT[:1, 0, :1], 0.0)
        out_blocks = range(QB) if dbg_stage in ("", "attn", "full") else []
        # output projection + bias
        with (
            tc.tile_pool(name="z", bufs=2) as zpool,
            tc.tile_pool(name="psZ", bufs=2, space="PSUM") as psZpool,
        ):
            for i in out_blocks:
                zt = zpool.tile([P, C], f32, tag="z")
                for nc2 in range(2):
                    ps = psZpool.tile([P, 512], f32, tag="z")
                    for cb in range(8):
                        nc.tensor.matmul(
                            ps,
                            lhsT=YT[:, cb, i * P : (i + 1) * P],
                            rhs=WO[:, cb, 512 * nc2 : 512 * (nc2 + 1)],
                            start=(cb == 0),
                            stop=False,
                        )
                    nc.tensor.matmul(
                        ps,
                        lhsT=on1,
                        rhs=bo[:, 512 * nc2 : 512 * (nc2 + 1)],
                        start=False,
                        stop=True,
                    )
                    nc.vector.tensor_copy(
                        out=zt[:, 512 * nc2 : 512 * (nc2 + 1)], in_=ps
                    )
                nc.sync.dma_start(out=yr_d[:, i], in_=zt)

    split_sync_waits(nc)
    return nc


def _host_inputs(x, w_qkv, b_qkv, w_out, b_out):
    x = np.asarray(x, dtype=np.float32)
    w_qkv = np.asarray(w_qkv, dtype=np.float32)
    b_qkv = np.asarray(b_qkv, dtype=np.float32)
    w_out = np.asarray(w_out, dtype=np.float32)
    b_out = np.asarray(b_out, dtype=np.float32)

    wqkv_r = np.ascontiguousarray(
        w_qkv[:, :2048].reshape(8, P, 16, P).transpose(2, 1, 0, 3)
    ).astype(np.float16)
    wv_r = np.ascontiguousarray(
        w_qkv[:, 2048:].reshape(8, P, 1024).transpose(1, 0, 2)
    ).astype(np.float16)
    wout_r = np.ascontiguousarray(
        w_out.reshape(8, P, C).transpose(1, 0, 2)
    ).astype(np.float16)
    bqkv_r = np.ascontiguousarray(b_qkv[:2048].reshape(16, P).T)
    bv_r = np.ascontiguousarray(
        np.broadcast_to(b_qkv[2048:], (P, 1024))
    ).astype(np.float16)
    bout_r = b_out.reshape(1, C).astype(np.float16)
    tri = np.triu(np.ones((P, P), dtype=np.float16))  # [k, q]: k <= q
    zer = np.zeros((P, P), dtype=np.float16)
    one = np.ones((P, P), dtype=np.float16)
    ones1 = np.ones((1, P), dtype=np.float16)

    in_maps = []
    for c in range(N_CORES):
        b, par = c // 2, c % 2
        xb = x[b]
        xT = np.ascontiguousarray(xb.T).astype(np.float16)
        qg = [2 * i + par for i in range(QB)]
        xq = np.concatenate([xb[g * P : (g + 1) * P] for g in qg], axis=0)
        xqT = np.ascontiguousarray(xq.T).astype(np.float16)
        # first-valid-block mask per kb: the first q block i0 = kb//2 has
        # g0 = 2*(kb//2) + par; g0 == kb -> tri, g0 < kb -> zeros (wasted
        # block for the other parity), g0 > kb -> ones (fully valid).
        mk = np.empty((P, KB, P), dtype=np.float16)
        for kb in range(KB):
            g0 = 2 * (kb // 2) + par
            mk[:, kb, :] = tri if g0 == kb else (zer if g0 < kb else one)
        in_maps.append(
            {
                "xt": xT,
                "xqt": xqT,
                "wqkv": wqkv_r,
                "wv": wv_r,
                "wout": wout_r,
                "bqkv": bqkv_r,
                "bv128": bv_r,
                "bout": bout_r,
                "msk": mk,
                "ones1": ones1,
            }
        )
    return in_maps


def kernel(x, w_qkv, b_qkv, w_out, b_out, trace=False):
    global _COMPILED, LAST_EXEC_NS, LAST_TRACE_PATH
    from concourse import bass_utils

    if _COMPILED is None:
        _COMPILED = _build()
    nc = _COMPILED

    in_maps = _host_inputs(x, w_qkv, b_qkv, w_out, b_out)
    res = bass_utils.run_bass_kernel_spmd(
        nc, in_maps, core_ids=list(range(N_CORES)), trace=trace
    )
    LAST_EXEC_NS = res.exec_time_ns
    if res.instructions_and_trace:
        LAST_TRACE_PATH = res.instructions_and_trace[1]

    y = np.empty((B, T, C), dtype=np.float32)
    for c in range(N_CORES):
        b, par = c // 2, c % 2
        yl = res.results[c]["yr"].transpose(1, 0, 2)  # [QB, P, C]
        for i in range(QB):
            g = 2 * i + par
            y[b, g * P : (g + 1) * P] = yl[i]
    return y


# revision 20
# speedup vs baseline: 1.1261x; 1.1261x over previous
"""Causal self-attention Trainium2 kernel (B=4, T=2048, C=1024, H=16, D=64).

Sharding: 8 cores = 4 batches x 2 causally-balanced query shards.
Core c handles batch b=c//2 and the 8 interleaved query blocks
g = 2*i + (c%2), i in 0..7 (block = 128 rows).  Every core computes full-
sequence K/V for its batch plus Q for its own query rows, runs all 16 heads
of attention for those rows, and the full output projection for them, so
per-core outputs are disjoint row-slices of y: no collectives, no host math.
One program serves both query parities: the per-kb first-block mask input
(tri / zeros / ones) encodes the parity-dependent causal structure.

Device-side dataflow (matmuls fp16 in / fp32 PSUM accumulate).  All matmul
streams are >=~512 columns so LDWEIGHTS hides behind the moving-tensor
stream (a previous 65/128-col structure was weight-load bound):
  Q^T,K^T = w^T @ x^T      (w stationary; [d, t] layout, head pairs stacked)
  V       = x^T.T @ w_v    (x^T chunks stationary, w_v moving -> V in
                            natural [t, c] layout, no PE transpose)
  S^T[kb] = K[kb] @ Q^T    (K block stationary, streams the whole causally
                            valid q-tail; two heads on partition halves)
  P^T     = exp(0.125*S^T) (ACT, psum->sbuf fp16), data mask on the first
                            valid q-block per kb
  Y^T     = [V|1].T @ P^T  (V natural stationary; accumulates [65, 1024]
                            over kb; row 64 = softmax denominators)
  YT      = Y^T * bcast(1/row64)  (PE ones-matmul broadcast + DVE mul)
  Z       = YT.T @ w_out + ones x b_out  (bias as a K=1 matmul)
"""

import os

import numpy as np

B, T, C = 4, 2048, 1024
H, D = 16, 64
N_CORES = 8
P = 128
QB = 8  # local query blocks per core (of 128 rows)
KB = 16  # key blocks per sequence
PAIRS = 8  # head pairs

_COMPILED = None
LAST_EXEC_NS = None
LAST_TRACE_PATH = None

dbg_stage = os.environ.get("KERNEL_DEBUG_STAGE", "")


def _get_mybir():
    import concourse.mybir as mybir
    return mybir


def split_sync_waits(nc):
    mybir = _get_mybir()
    # This walrus build rejects instructions carrying more than one sync
    # wait (or update).  Split the extras onto NOP carriers: waits go on
    # NOPs inserted before the instruction (same engine, so they gate it),
    # updates onto NOPs after it (fire once it has completed).
    uid = [0]

    def carrier(engine, wait=None, update=None):
        uid[0] += 1
        n = mybir.InstNoOp(
            name=f"I-syncsplit-{uid[0]}",
            opcode="NoOp",
            ins=[],
            outs=[],
            sync_info=mybir.SyncInfo(
                on_wait=[wait] if wait else [],
                on_update=[update] if update else [],
            ),
        )
        n.engine = engine
        return n

    for f in nc.m.functions:
        for blk in f.blocks:
            out = []
            changed = False
            for inst in blk.instructions:
                si = inst.sync_info
                if si is None or (
                    len(si.on_wait) <= 1 and len(si.on_update) <= 1
                ):
                    out.append(inst)
                    continue
                changed = True
                waits = list(si.on_wait)
                updates = list(si.on_update)
                for w in waits[1:]:
                    out.append(carrier(inst.engine, wait=w))
                inst.sync_info = mybir.SyncInfo(
                    on_wait=waits[:1], on_update=updates[:1]
                )
                out.append(inst)
                for u in updates[1:]:
                    out.append(carrier(inst.engine, update=u))
            if changed:
                blk.instructions = out


def _build():
    import concourse.bass as bass
    import concourse.tile as tile
    import concourse.mybir as mybir
    from contextlib import ExitStack

    f32 = mybir.dt.float32
    f16 = mybir.dt.float16
    AF = mybir.ActivationFunctionType

    nc = bass.Bass(
        "TRN2", target_bir_lowering=False, debug=False, num_devices=N_CORES
    )

    def act_recip(out, in_):
        # ACT-engine reciprocal (measured max rel err ~9e-4 on [0.5, 3e4],
        # plenty for softmax normalization).  The bass wrapper refuses
        # AF.Reciprocal outright, so emit the InstActivation directly.
        eng = nc.scalar
        inputs = [eng.lower_ap(in_)]
        for arg in (0.0, 1.0, 0.0):  # bias, scale, alpha
            inputs.append(mybir.ImmediateValue(dtype=mybir.dt.float32, value=arg))
        return eng.add_instruction(
            mybir.InstActivation(
                name=nc.get_next_instruction_name(),
                func=AF.Reciprocal,
                ins=inputs,
                outs=[eng.lower_ap(out)],
            )
        )

    xT_d = nc.dram_tensor("xt", [C, T], f16, kind="ExternalInput").ap()
    xqT_d = nc.dram_tensor("xqt", [C, QB * P], f16, kind="ExternalInput").ap()
    wqkv_d = nc.dram_tensor("wqkv", [16, P, 8, P], f16, kind="ExternalInput").ap()
    wv_d = nc.dram_tensor("wv", [P, 8, 1024], f16, kind="ExternalInput").ap()
    wout_d = nc.dram_tensor("wout", [P, 8, C], f16, kind="ExternalInput").ap()
    bqkv_d = nc.dram_tensor("bqkv", [P, 16], f32, kind="ExternalInput").ap()
    bv_d = nc.dram_tensor("bv128", [P, 1024], f16, kind="ExternalInput").ap()
    bout_d = nc.dram_tensor("bout", [1, C], f16, kind="ExternalInput").ap()
    msk_d = nc.dram_tensor("msk", [P, KB, P], f16, kind="ExternalInput").ap()
    ones_d = nc.dram_tensor("ones1", [1, P], f16, kind="ExternalInput").ap()
    yr_d = nc.dram_tensor("yr", [P, QB, C], f32, kind="ExternalOutput").ap()
    dbg_d = {}
    if dbg_stage in ("proj", "vn", "attn"):
        dbg_d["kt"] = nc.dram_tensor(
            "kt", [P, PAIRS, T], f16, kind="ExternalOutput"
        ).ap()
        dbg_d["qt"] = nc.dram_tensor(
            "qt", [P, PAIRS, QB * P], f16, kind="ExternalOutput"
        ).ap()
    if dbg_stage in ("vn", "attn"):
        dbg_d["vn"] = nc.dram_tensor(
            "vn", [P, PAIRS, KB, 2, 65], f16, kind="ExternalOutput"
        ).ap()
    if dbg_stage == "attn":
        dbg_d["yt"] = nc.dram_tensor(
            "yt", [P, 8, QB * P], f16, kind="ExternalOutput"
        ).ap()

    with tile.TileContext(nc) as tc, ExitStack() as ctx:
        persist = ctx.enter_context(tc.tile_pool(name="persist", bufs=1))
        KT = persist.tile([P, PAIRS, T], f16)
        QT = persist.tile([P, PAIRS, QB * P], f16)
        YT = persist.tile([P, 8, QB * P], f16)
        VN = persist.tile([P, PAIRS, KB, 2, 65], f16)
        WO = persist.tile([P, 8, C], f16)
        msk = persist.tile([P, KB, P], f16)
        bqs = persist.tile([P, 16], f32)
        bv = persist.tile([P, 1024], f16)
        on1 = persist.tile([1, P], f16)
        bo = persist.tile([1, C], f16)

        # DMA issue order = need order: bqs (first psum->sbuf copies), the
        # first Q-proj weight block, XQT, XT, then weights and attention-
        # phase constants.  Single-start strided transfers keep the sync
        # engine's per-dma issue cost (~0.6us) off the critical path.
        xT_v = xT_d.rearrange("(cb p) t -> p cb t", p=P)
        xqT_v = xqT_d.rearrange("(cb p) t -> p cb t", p=P)

        wpool = ctx.enter_context(tc.tile_pool(name="w", bufs=3))
        wt0 = wpool.tile([P, 8, P], f16, tag="w")
        nc.sync.dma_start(out=bqs, in_=bqkv_d)
        nc.sync.dma_start(out=wt0, in_=wqkv_d[0])

        with (
            tc.tile_pool(name="xt", bufs=1) as xtpool,
            tc.tile_pool(name="xqt", bufs=1) as xqtpool,
            tc.tile_pool(name="wv", bufs=1) as wvpool,
            tc.tile_pool(name="psproj", bufs=3, space="PSUM") as pspool,
        ):
            XQT = xqtpool.tile([P, 8, QB * P], f16)
            nc.sync.dma_start(out=XQT, in_=xqT_v)
            XT = xtpool.tile([P, 8, T], f16)
            nc.sync.dma_start(out=XT, in_=xT_v)
            WV = wvpool.tile([P, 8, 1024], f16)
            nc.sync.dma_start(out=WV, in_=wv_d)
            nc.sync.dma_start(out=WO, in_=wout_d)
            nc.sync.dma_start(out=msk, in_=msk_d)
            nc.sync.dma_start(out=bv, in_=bv_d)
            nc.sync.dma_start(out=on1, in_=ones_d)
            nc.sync.dma_start(out=bo, in_=bout_d)
            nc.vector.memset(VN[:, :, :, :, 64:65], 1.0)

            # Q^T projection (j-blocks 0..7): w stationary, XQT moving
            for pb in range(PAIRS):
                if pb == 0:
                    wt = wt0
                else:
                    wt = wpool.tile([P, 8, P], f16, tag="w")
                    nc.sync.dma_start(out=wt, in_=wqkv_d[pb])
                for t4 in range(2):
                    ps = pspool.tile([P, 512], f32, tag="proj")
                    for cb in range(8):
                        nc.tensor.matmul(
                            ps,
                            lhsT=wt[:, cb],
                            rhs=XQT[:, cb, 512 * t4 : 512 * (t4 + 1)],
                            start=(cb == 0),
                            stop=(cb == 7),
                        )
                    nc.vector.tensor_scalar_add(
                        QT[:, pb, 512 * t4 : 512 * (t4 + 1)],
                        ps,
                        bqs[:, pb : pb + 1],
                    )

            # K^T projection (j-blocks 8..15) over full T
            for pb in range(PAIRS):
                wt = wpool.tile([P, 8, P], f16, tag="w")
                nc.sync.dma_start(out=wt, in_=wqkv_d[8 + pb])
                for t4 in range(4):
                    ps = pspool.tile([P, 512], f32, tag="proj")
                    for cb in range(8):
                        nc.tensor.matmul(
                            ps,
                            lhsT=wt[:, cb],
                            rhs=XT[:, cb, 512 * t4 : 512 * (t4 + 1)],
                            start=(cb == 0),
                            stop=(cb == 7),
                        )
                    nc.vector.tensor_scalar_add(
                        KT[:, pb, 512 * t4 : 512 * (t4 + 1)],
                        ps,
                        bqs[:, 8 + pb : 9 + pb],
                    )

            # V in natural [t, c] layout: x^T chunk stationary, w_v moving
            if dbg_stage != "proj":
                for kb in range(KB):
                    for half in range(2):
                        ps = pspool.tile([P, 512], f32, tag="proj")
                        for cb in range(8):
                            nc.tensor.matmul(
                                ps,
                                lhsT=XT[:, cb, kb * P : (kb + 1) * P],
                                rhs=WV[:, cb, 512 * half : 512 * (half + 1)],
                                start=(cb == 0),
                                stop=(cb == 7),
                            )
                        nc.vector.tensor_add(
                            out=VN[:, 4 * half : 4 * half + 4, kb, :, 0:64],
                            in0=ps.rearrange("p (a b c) -> p a b c", b=2, c=64),
                            in1=bv[:, 512 * half : 512 * (half + 1)].rearrange(
                                "p (a b c) -> p a b c", b=2, c=64
                            ),
                        )

        if "kt" in dbg_d:
            nc.sync.dma_start(out=dbg_d["kt"], in_=KT)
            nc.sync.dma_start(out=dbg_d["qt"], in_=QT)
        if "vn" in dbg_d:
            nc.sync.dma_start(out=dbg_d["vn"], in_=VN)

        # attention: per (pb): S^T per key block over the causally valid
        # q-tail, exp, first-block data mask, then V-stationary accumulation
        # of Y^T[65, 1024] (row 64 = softmax sums), normalized into YT.
        attn_pairs = range(PAIRS) if dbg_stage in ("", "attn", "full") else []
        DEPTH = 3  # S-units emitted ahead of their Y-matmul (hides ACT exp)
        with (
            tc.tile_pool(name="pts", bufs=DEPTH + 1) as ptpool,
            tc.tile_pool(name="sm", bufs=2) as smpool,
            tc.tile_pool(name="psS", bufs=DEPTH + 1, space="PSUM") as psSpool,
            tc.tile_pool(name="psY", bufs=1, space="PSUM") as psYpool,
        ):
            for pb in attn_pairs:
                psYT = [
                    psYpool.tile([65, QB * P], f32, tag=f"y{h}", name=f"psYT{h}")
                    for h in range(2)
                ]
                # units in accumulation order; Y(u) trails S(u) by DEPTH so
                # the PE never waits on ACT exp (keeps the pstate ramped).
                units = []
                for kb in range(KB):
                    q0 = (kb // 2) * P
                    chunks = (
                        [(q0, 512), (512, 1024)] if q0 < 512 else [(q0, 1024)]
                    )
                    for (c0, c1) in chunks:
                        for h in range(2):
                            last = (kb == 7) if c1 <= 512 else (kb == KB - 1)
                            units.append((kb, c0, c1, h, q0, last))

                pend = []

                def emit_y(u, pt):
                    kb, c0, c1, h, q0, last = u
                    nc.tensor.matmul(
                        psYT[h][:, c0:c1],
                        lhsT=VN[:, pb, kb, h],
                        rhs=pt,
                        start=(kb == 0),
                        stop=last,
                        skip_group_check=True,
                    )

                for u in units:
                    kb, c0, c1, h, q0, last = u
                    cw = c1 - c0
                    psS = psSpool.tile([P, 512], f32, tag="s", name="psS")[
                        :, :cw
                    ]
                    nc.tensor.matmul(
                        psS,
                        lhsT=KT[
                            64 * h : 64 * h + 64, pb, kb * P : (kb + 1) * P
                        ],
                        rhs=QT[64 * h : 64 * h + 64, pb, c0:c1],
                        start=True,
                        stop=True,
                    )
                    pt = ptpool.tile([P, 512], f16, tag="pt", name="pt")[
                        :, :cw
                    ]
                    nc.scalar.activation(pt, psS, AF.Exp, scale=0.125)
                    if c0 <= q0 < c1:
                        nc.vector.tensor_mul(
                            out=pt[:, q0 - c0 : q0 - c0 + P],
                            in0=pt[:, q0 - c0 : q0 - c0 + P],
                            in1=msk[:, kb, :],
                        )
                    pend.append((u, pt))
                    if len(pend) > DEPTH:
                        emit_y(*pend.pop(0))
                for item in pend:
                    emit_y(*item)
                pend = []

                # normalize: broadcast raw sums (row 64) via a K=1 ones
                # matmul, reciprocal on ACT (128-lane), multiply on DVE.
                for h in range(2):
                    su = smpool.tile([1, QB * P], f16, tag=f"su{h}")
                    nc.scalar.copy(su, psYT[h][64:65, :])
                    for half in range(2):
                        rb = psSpool.tile([P, 512], f32, tag="s", name="rb")
                        nc.tensor.matmul(
                            rb[0:64, :],
                            lhsT=on1[:, 0:64],
                            rhs=su[:, 512 * half : 512 * (half + 1)],
                            start=True,
                            stop=True,
                        )
                        rs = smpool.tile([64, 512], f16, tag=f"rs{h}")
                        act_recip(rs, rb[0:64, :])
                        nc.vector.tensor_mul(
                            out=YT[
                                64 * h : 64 * h + 64,
                                pb,
                                512 * half : 512 * (half + 1),
                            ],
                            in0=psYT[h][0:64, 512 * half : 512 * (half + 1)],
                            in1=rs,
                        )

        if "yt" in dbg_d:
            nc.sync.dma_start(out=dbg_d["yt"], in_=YT)
        if dbg_stage:
            nc.vector.memset(YT[:1, 0, :1], 0.0)
        out_blocks = range(QB) if dbg_stage in ("", "attn", "full") else []
        # output projection + bias
        with (
            tc.tile_pool(name="z", bufs=2) as zpool,
            tc.tile_pool(name="psZ", bufs=2, space="PSUM") as psZpool,
        ):
            for i in out_blocks:
                zt = zpool.tile([P, C], f32, tag="z")
                for nc2 in range(2):
                    ps = psZpool.tile([P, 512], f32, tag="z")
                    for cb in range(8):
                        nc.tensor.matmul(
                            ps,
                            lhsT=YT[:, cb, i * P : (i + 1) * P],
                            rhs=WO[:, cb, 512 * nc2 : 512 * (nc2 + 1)],
                            start=(cb == 0),
                            stop=False,
                        )
                    nc.tensor.matmul(
                        ps,
                        lhsT=on1,
                        rhs=bo[:, 512 * nc2 : 512 * (nc2 + 1)],
                        start=False,
                        stop=True,
                    )
                    nc.vector.tensor_copy(
                        out=zt[:, 512 * nc2 : 512 * (nc2 + 1)], in_=ps
                    )
                nc.sync.dma_start(out=yr_d[:, i], in_=zt)

    split_sync_waits(nc)
    return nc


def _host_inputs(x, w_qkv, b_qkv, w_out, b_out):
    x = np.asarray(x, dtype=np.float32)
    w_qkv = np.asarray(w_qkv, dtype=np.float32)
    b_qkv = np.asarray(b_qkv, dtype=np.float32)
    w_out = np.asarray(w_out, dtype=np.float32)
    b_out = np.asarray(b_out, dtype=np.float32)

    wqkv_r = np.ascontiguousarray(
        w_qkv[:, :2048].reshape(8, P, 16, P).transpose(2, 1, 0, 3)
    ).astype(np.float16)
    wv_r = np.ascontiguousarray(
        w_qkv[:, 2048:].reshape(8, P, 1024).transpose(1, 0, 2)
    ).astype(np.float16)
    wout_r = np.ascontiguousarray(
        w_out.reshape(8, P, C).transpose(1, 0, 2)
    ).astype(np.float16)
    bqkv_r = np.ascontiguousarray(b_qkv[:2048].reshape(16, P).T)
    bv_r = np.ascontiguousarray(
        np.broadcast_to(b_qkv[2048:], (P, 1024))
    ).astype(np.float16)
    bout_r = b_out.reshape(1, C).astype(np.float16)
    tri = np.triu(np.ones((P, P), dtype=np.float16))  # [k, q]: k <= q
    zer = np.zeros((P, P), dtype=np.float16)
    one = np.ones((P, P), dtype=np.float16)
    ones1 = np.ones((1, P), dtype=np.float16)

    in_maps = []
    for c in range(N_CORES):
        b, par = c // 2, c % 2
        xb = x[b]
        xT = np.ascontiguousarray(xb.T).astype(np.float16)
        qg = [2 * i + par for i in range(QB)]
        xq = np.concatenate([xb[g * P : (g + 1) * P] for g in qg], axis=0)
        xqT = np.ascontiguousarray(xq.T).astype(np.float16)
        # first-valid-block mask per kb: the first q block i0 = kb//2 has
        # g0 = 2*(kb//2) + par; g0 == kb -> tri, g0 < kb -> zeros (wasted
        # block for the other parity), g0 > kb -> ones (fully valid).
        mk = np.empty((P, KB, P), dtype=np.float16)
        for kb in range(KB):
            g0 = 2 * (kb // 2) + par
            mk[:, kb, :] = tri if g0 == kb else (zer if g0 < kb else one)
        in_maps.append(
            {
                "xt": xT,
                "xqt": xqT,
                "wqkv": wqkv_r,
                "wv": wv_r,
                "wout": wout_r,
                "bqkv": bqkv_r,
                "bv128": bv_r,
                "bout": bout_r,
                "msk": mk,
                "ones1": ones1,
            }
        )
    return in_maps


def kernel(x, w_qkv, b_qkv, w_out, b_out, trace=False):
    global _COMPILED, LAST_EXEC_NS, LAST_TRACE_PATH
    from concourse import bass_utils

    if _COMPILED is None:
        _COMPILED = _build()
    nc = _COMPILED

    in_maps = _host_inputs(x, w_qkv, b_qkv, w_out, b_out)
    res = bass_utils.run_bass_kernel_spmd(
        nc, in_maps, core_ids=list(range(N_CORES)), trace=trace
    )
    LAST_EXEC_NS = res.exec_time_ns
    if res.instructions_and_trace:
        LAST_TRACE_PATH = res.instructions_and_trace[1]

    y = np.empty((B, T, C), dtype=np.float32)
    for c in range(N_CORES):
        b, par = c // 2, c % 2
        yl = res.results[c]["yr"].transpose(1, 0, 2)  # [QB, P, C]
        for i in range(QB):
            g = 2 * i + par
            y[b, g * P : (g + 1) * P] = yl[i]
    return y
